# revision 18
# baseline (speedup 1.0000x reference)
"""Trainium2 Bass kernel for nn_DARTSModelLayers (FISTA-style unrolled model).

Math (per reference):
  W = frozen_weight[0]  [N=512, H=1024];  L = ||W||_2^2;  lam = 0.001/L
  10 iterations of:
    z_aux = z + (i/(i+3)) (z - z_prev)
    z_g   = z_aux - W^T(W z_aux - x)/L  =  (I - W^T W / L) z_aux + W^T x / L
    z_op  = sum_k softmax(alpha_i)_k * op_k(z_g)        (20 activations)
    z_prev = bw0 z + bw1 z_op ; z = z_op
  Re-expressed with host-folded scalars so each iteration is:
    tmp  = z_op_{i-2} * (coef_zold/coef_op) + z_op_{i-1}   (1 DVE pass)
    psum = M_noI @ tmp + I @ tmp                            (PE; M = I + M_noI)
    z_g  = psum * coef_op + c'                              (1 DVE pass)
    z_op = S_i(z_g)   via basis decomposition (ACT LUTs + DVE chains)

  S_i decomposition (weights w = softmax(alpha_i), see golden.py):
    basis: sigmoid(-x), tanh(x), erf(x/sqrt2), ln(sigmoid(-x)), exp(min(x,0)),
           sigmoid(-x)^2, |x|, clip(x,-1,1), clip(x/6+.5,0,1),
           1/(1+sm^2), 1/(1+|x|)
    z_op = x*V + sum_k cW_k * U_k, V = c_v0 + sum_k cV_k * T_k
    softshrink/hardshrink are approximated by identity (lam ~ 3.5e-4; max
    output error ~1e-4, validated against the jax reference in golden.py).

Sharding: batch B=4096 split over 8 cores (512 each); W/alpha/beta replicated.
M_noI = -W^T W/L and c' = W^T x/L are computed on-device; the host only
supplies the spectral norm L, softmax weights, and identity constants.
Output is produced in [H, B_shard] layout; the host transposes to [B, H, 1].
"""
import sys
import numpy as np

sys.path.insert(0, "/opt/trn_rl_repo")

import concourse.bass as bass  # noqa: E402
import concourse.bacc as bacc  # noqa: E402
import concourse.tile as tile  # noqa: E402
from concourse import mybir  # noqa: E402
from concourse.bass_utils import run_bass_kernel_spmd  # noqa: E402
from contextlib import ExitStack  # noqa: E402

F32 = mybir.dt.float32
F32R = mybir.dt.float32r
ACT = mybir.ActivationFunctionType
ALU = mybir.AluOpType

B, N, H, T = 4096, 512, 1024, 10
NCORES = 8
BS = B // NCORES          # 512 batch per core
NG = H // 128             # 8 h-tile groups
INV_SQRT2 = 0.7071067811865476
LAM_SELU = 1.0507009873554805
ALPHA_SELU = 1.6732632423543772

# ---- tuning switches ----
MM_DT = mybir.dt.float16      # per-iteration matmul dtype
SETUP_MM_DT = mybir.dt.float16  # setup (W^T W, W^T x) matmul dtype
CHAIN_DT = mybir.dt.float16   # basis/elementwise chain dtype
ACT_BATCH = 8                 # groups per ACT table-set batch
TRACE = False                 # set by test harness
DEBUG_DUMP = False            # add intermediate ExternalOutputs


def _store_dt(dt):
    # float32r tiles are real-typed: the BIR verifier requires matmul inputs
    # to be produced (rounded) as float32r, so no bitcasting.
    return dt


def _softmax(v):
    v = v - v.max()
    e = np.exp(v)
    return e / e.sum()


def _build(L, aw, bw, t_override=None):
    """Build the Bass program. aw [T,20], bw [T,2] host floats."""
    nc = bacc.Bacc("TRN2", target_bir_lowering=False, debug=False,
                   num_devices=NCORES)
    mm_st = _store_dt(MM_DT)
    su_st = _store_dt(SETUP_MM_DT)

    x_d = nc.dram_tensor("x", [BS, N], F32, kind="ExternalInput")
    w_d = nc.dram_tensor("w", [N, H], F32, kind="ExternalInput")
    im_d = nc.dram_tensor("ident_mm", [128, 128], mm_st, kind="ExternalInput")
    z_d = nc.dram_tensor("z_out", [H, BS], F32, kind="ExternalOutput")
    dbg = {}
    if DEBUG_DUMP:
        for nm, shp in (("c_sb", [128, NG * BS]), ("m_sb", [128, NG * H]),
                        ("z0", [128, NG * BS]), ("zg1", [128, NG * BS]),
                        ("xT", [128, 4 * BS])):
            dt = _store_dt(MM_DT) if nm == "m_sb" else F32
            dt = _store_dt(SETUP_MM_DT) if nm == "xT" else dt
            dbg[nm] = nc.dram_tensor("dbg_" + nm, shp, dt,
                                     kind="ExternalOutput")

    invL = 1.0 / L

    with tile.TileContext(nc) as tc, ExitStack() as ctx:
        ctx.enter_context(nc.allow_low_precision(
            reason="fp16 basis chain; error validated against jax reference"))
        state = ctx.enter_context(tc.tile_pool(name="state", bufs=1))
        # 8 persistent psum tiles (one bank each) -- no pool cycling, so no
        # SP release waits land on matmuls (walrus: max 1 wait per Matmult)
        psfix = ctx.enter_context(tc.tile_pool(name="psfix", bufs=1,
                                               space="PSUM"))
        ps_fix = [psfix.tile([128, BS], F32, name=f"psf{g}") for g in range(NG)]
        zA = state.tile([128, NG * BS], F32, name="zA")
        zB = state.tile([128, NG * BS], F32, name="zB")
        zg = state.tile([128, NG * BS], F32, name="zg")
        c_sb = state.tile([128, NG * BS], F32, name="c_sb")
        m_sb = state.tile([128, NG * H], mm_st, name="m_sb")
        ident_mm = state.tile([128, 128], mm_st, name="ident_mm")
        nc.sync.dma_start(ident_mm[:], im_d[:, :])

        # ---------------- setup: M_noI and c' ----------------
        with tc.tile_pool(name="setup", bufs=1) as sp:
            w_sb = sp.tile([128, 4 * H], F32, name="w_sb")
            w_rhs = sp.tile([128, 4 * H], su_st, name="w_rhs")
            w_lhs = sp.tile([128, 4 * H], su_st, name="w_lhs")
            x_sb = sp.tile([128, 4 * N], F32, name="x_sb")
            x16 = sp.tile([128, 4 * N], F32, name="x16")
            xT_sb = sp.tile([128, 4 * BS], su_st, name="xT_sb")
            xT2 = sp.tile([128, 4 * BS], su_st, name="xT2")
            # single DMA per tensor so downstream consumers carry one wait
            nc.sync.dma_start(w_sb[:].rearrange("p (j h) -> p j h", j=4),
                              w_d[:, :].rearrange("(j p) h -> p j h", p=128))
            nc.sync.dma_start(x_sb[:].rearrange("p (j n) -> p j n", j=4),
                              x_d[:, :].rearrange("(j p) n -> p j n", p=128))
            # rhs for A-matmul: -W/L (DVE); lhs copy of W (ACT)
            nc.vector.tensor_scalar(w_rhs[:], w_sb[:], -invL, None, ALU.mult)
            nc.scalar.copy(w_lhs[:], w_sb[:])
            # x/L in f32, PE-transpose 128x128 blocks to [n, b] layout, cast
            # to fp16 on evacuation. Both transpose inputs come from ACT so
            # each transpose carries a single (ACT) wait.
            nc.scalar.activation(x16[:], x_sb[:], ACT.Copy, scale=invL)
            identf_a = sp.tile([128, 128], F32, name="identf_a")
            nc.scalar.copy(identf_a[:], ident_mm[:])
            for bj in range(4):
                for nk in range(4):
                    pst = ps_fix[bj]
                    nc.tensor.transpose(
                        pst[:, 0:128],
                        x16[:, bj * N + nk * 128: bj * N + nk * 128 + 128],
                        identf_a[:])
                    nc.vector.tensor_scalar(
                        xT_sb[:, nk * BS + bj * 128: nk * BS + bj * 128 + 128],
                        pst[:, 0:128], 1.0, None, ALU.mult)
            # funnel the 16 transpose evacs through one ACT producer so the
            # cc-matmuls carry a single (ACT) wait
            nc.scalar.copy(xT2[:], xT_sb[:])
            if DEBUG_DUMP:
                nc.sync.dma_start(dbg["xT"][:, :], xT2[:])

            # M_noI = -W^T W / L   -> m_sb[h1-part(g), h2-free]
            for g in range(NG):
                for half in range(2):
                    ps = ps_fix[g]
                    for j in range(4):
                        nc.tensor.matmul(
                            ps[:, half * 0: 512],
                            w_lhs[:, j * H + g * 128: j * H + g * 128 + 128],
                            w_rhs[:, j * H + half * 512: j * H + half * 512 + 512],
                            start=(j == 0), stop=(j == 3))
                    nc.scalar.copy(
                        m_sb[:, g * H + half * 512: g * H + half * 512 + 512],
                        ps[:, 0:512])

            # c' = W^T x / L  -> c_sb[h-part(g), b-free]
            for g in range(NG):
                ps = ps_fix[g]
                for nk in range(4):
                    nc.tensor.matmul(
                        ps[:],
                        w_lhs[:, nk * H + g * 128: nk * H + g * 128 + 128],
                        xT2[:, nk * BS:(nk + 1) * BS],
                        start=(nk == 0), stop=(nk == 3))
                # evacuate via DVE so iteration-1 matmuls' WAR on this bank
                # merges with their DVE wait on tmp
                nc.vector.tensor_scalar(c_sb[:, g * BS:(g + 1) * BS], ps[:],
                                        1.0, None, ALU.mult)

            nc.vector.memset(zB[:], 0.0)

        # ---------------- iterations ----------------
        actp = ctx.enter_context(tc.tile_pool(name="actb", bufs=ACT_BATCH))
        dvep = ctx.enter_context(tc.tile_pool(name="dveb", bufs=2))
        tmp = state.tile([128, NG * BS], mm_st, name="tmp")
        xh_t = (state.tile([128, NG * BS], CHAIN_DT, name="xh_t")
                if CHAIN_DT != F32 else None)

        z_im1, z_im2 = None, zB   # z_op_{i-1}, z_op_{i-2}
        T_eff = T if t_override is None else t_override
        for i in range(T_eff):
            w = aw[i]
            c_r = w[1] + 0.99 * w[10] + w[4] + w[9] + LAM_SELU * w[8]
            wE = w[4] + w[9] + LAM_SELU * ALPHA_SELU * w[8]
            K = w[16] - wE
            c_v0 = (w[2] + w[11] + w[12] + 0.5 * w[3] + w[18] - w[19]
                    + 0.01 * w[10] + 0.5 * c_r + w[0] + w[5])
            cV = {"e2": 0.5 * w[3], "sm": -w[18], "r1": 2.0 * w[19],
                  "r2": w[13], "hm": w[7]}
            cW = {"A": 0.5 * c_r, "t": w[15] - w[12], "sm": -w[16],
                  "ln": w[11] - w[14], "E": wE, "c1": w[6], "hm": w[17]}

            if i == 0:
                x_src = c_sb
            else:
                mom = i / (i + 3.0)
                bwp = bw[i - 1]
                coef_op = 1.0 + mom * (1.0 - bwp[1])
                coef_zold = -mom * bwp[0]
                nc.vector.scalar_tensor_tensor(
                    tmp[:], z_im2[:], coef_zold / coef_op, z_im1[:],
                    ALU.mult, ALU.add)
                for g in range(NG):
                    ps = ps_fix[g]
                    for j in range(NG):
                        nc.tensor.matmul(
                            ps[:],
                            m_sb[:, j * H + g * 128: j * H + g * 128 + 128],
                            tmp[:, j * BS:(j + 1) * BS],
                            start=(j == 0), stop=False)
                    nc.tensor.matmul(ps[:], ident_mm[:],
                                     tmp[:, g * BS:(g + 1) * BS],
                                     start=False, stop=True)
                    nc.vector.scalar_tensor_tensor(
                        zg[:, g * BS:(g + 1) * BS], ps[:], coef_op,
                        c_sb[:, g * BS:(g + 1) * BS], ALU.mult, ALU.add)
                x_src = zg

            z_out_t = zA if i % 2 == 0 else zB

            # fp16 copy of x for the DVE-side chain (ACT does the cast)
            if CHAIN_DT != F32:
                xh = xh_t
                for g in range(NG):
                    nc.scalar.copy(xh[:, g * BS:(g + 1) * BS],
                                   x_src[:, g * BS:(g + 1) * BS])
            else:
                xh = x_src

            for b0 in range(0, NG, ACT_BATCH):
                gr = list(range(b0, min(b0 + ACT_BATCH, NG)))

                def xs(g):
                    return x_src[:, g * BS:(g + 1) * BS]

                def xhs(g):
                    return xh[:, g * BS:(g + 1) * BS]

                # ACT batch 1 (sigmoid_and_others): sigmoid(-x), tanh, erf
                sm_l, t_l, e2_l, m0_l = {}, {}, {}, {}
                for g in gr:
                    sm_l[g] = actp.tile([128, BS], CHAIN_DT, tag="sm", name="sm")
                    nc.scalar.activation(sm_l[g][:], xs(g), ACT.Sigmoid, scale=-1.0)
                for g in gr:
                    t_l[g] = actp.tile([128, BS], CHAIN_DT, tag="t", name="t")
                    nc.scalar.activation(t_l[g][:], xs(g), ACT.Tanh)
                for g in gr:
                    e2_l[g] = actp.tile([128, BS], CHAIN_DT, tag="e2", name="e2")
                    nc.scalar.activation(e2_l[g][:], xs(g), ACT.Erf, scale=INV_SQRT2)
                for g in gr:
                    m0_l[g] = dvep.tile([128, BS], CHAIN_DT, tag="m0", name="m0",
                                        bufs=ACT_BATCH)
                    nc.vector.tensor_scalar(m0_l[g][:], xhs(g), 0.0, None, ALU.min)
                A_l = {}
                for g in gr:
                    A_l[g] = actp.tile([128, BS], CHAIN_DT, tag="Aq", name="Aq")
                    nc.scalar.activation(A_l[g][:], xs(g), ACT.Abs)
                # ACT batch 2 (natural_log_exp_and_others): ln(sm), sm^2, exp(m0)
                ln_l, s2_l, E_l = {}, {}, {}
                for g in gr:
                    ln_l[g] = actp.tile([128, BS], CHAIN_DT, tag="ln", name="ln")
                    nc.scalar.activation(ln_l[g][:], sm_l[g][:], ACT.Ln)
                for g in gr:
                    s2_l[g] = actp.tile([128, BS], CHAIN_DT, tag="s2", name="s2")
                    nc.scalar.activation(s2_l[g][:], sm_l[g][:], ACT.Square)
                for g in gr:
                    E_l[g] = actp.tile([128, BS], CHAIN_DT, tag="E", name="E")
                    nc.scalar.activation(E_l[g][:], m0_l[g][:], ACT.Exp)

                for g in gr:
                    def dv(tag):
                        return dvep.tile([128, BS], CHAIN_DT, tag=tag, name=tag)

                    Aq = A_l[g]
                    c1 = dv("c1")
                    nc.vector.tensor_scalar(c1[:], xhs(g), 1.0, -1.0,
                                            ALU.min, ALU.max)
                    c1k = dv("c1k")
                    nc.vector.tensor_scalar(c1k[:], c1[:], K / cW["c1"], None,
                                            ALU.add)
                    hm1 = dv("hm1")
                    nc.vector.tensor_scalar(hm1[:], xhs(g), 1.0 / 6.0, 0.5,
                                            ALU.mult, ALU.add)
                    hm = dv("hm")
                    nc.vector.tensor_scalar(hm[:], hm1[:], 1.0, 0.0,
                                            ALU.min, ALU.max)
                    d1 = dv("d1")
                    nc.vector.tensor_scalar(d1[:], s2_l[g][:], 1.0, None, ALU.add)
                    r1 = dv("r1")
                    nc.vector.reciprocal(r1[:], d1[:])
                    d2 = dv("d2")
                    nc.vector.tensor_scalar(d2[:], Aq[:], 1.0, None, ALU.add)
                    r2 = dv("r2")
                    nc.vector.reciprocal(r2[:], d2[:])

                    # V = c_e2*e2 + c_v0 + c_sm*sm + c_r1*r1 + c_r2*r2 + c_hm*hm
                    V = dv("V0")
                    nc.vector.tensor_scalar(V[:], e2_l[g][:], cV["e2"], c_v0,
                                            ALU.mult, ALU.add)
                    for tag, (tens, cc_) in zip(
                            ("V1", "V0", "V1", "V0"),
                            ((sm_l[g], cV["sm"]), (r1, cV["r1"]),
                             (r2, cV["r2"]), (hm, cV["hm"]))):
                        Vn = dv(tag)
                        nc.vector.scalar_tensor_tensor(Vn[:], tens[:], cc_, V[:],
                                                       ALU.mult, ALU.add)
                        V = Vn
                    acc = dv("acc0")
                    nc.vector.tensor_mul(acc[:], xhs(g), V[:])
                    chain = [(Aq, cW["A"]), (t_l[g], cW["t"]), (sm_l[g], cW["sm"]),
                             (ln_l[g], cW["ln"]), (E_l[g], cW["E"]),
                             (c1k, cW["c1"]), (hm, cW["hm"])]
                    for idx, (tens, cc_) in enumerate(chain):
                        if idx == len(chain) - 1:
                            nc.vector.scalar_tensor_tensor(
                                z_out_t[:, g * BS:(g + 1) * BS], tens[:], cc_,
                                acc[:], ALU.mult, ALU.add)
                        else:
                            nacc = dv("acc1" if idx % 2 == 0 else "acc0")
                            nc.vector.scalar_tensor_tensor(
                                nacc[:], tens[:], cc_, acc[:], ALU.mult, ALU.add)
                            acc = nacc

            z_im2 = z_im1 if z_im1 is not None else zB
            z_im1 = z_out_t

        if DEBUG_DUMP:
            nc.sync.dma_start(dbg["c_sb"][:, :], c_sb[:])
            nc.sync.dma_start(dbg["m_sb"][:, :], m_sb[:])
            nc.sync.dma_start(dbg["z0"][:, :], zA[:])
            nc.sync.dma_start(dbg["zg1"][:, :], zg[:])
        # output
        for g in range(NG):
            nc.sync.dma_start(z_d[g * 128:(g + 1) * 128, :],
                              z_im1[:, g * BS:(g + 1) * BS])

    nc.finalize()
    return nc


_CACHE = {}


def kernel(x, frozen_weight, alpha, layer_beta, _want_trace=False):
    x = np.asarray(x, np.float32)
    frozen_weight = np.asarray(frozen_weight, np.float32)
    alpha = np.asarray(alpha, np.float32)
    layer_beta = np.asarray(layer_beta, np.float32)

    W = frozen_weight[0]                                   # [N, H]
    L = float(np.linalg.norm(W.astype(np.float64), 2) ** 2)
    aw = np.stack([_softmax(alpha[i].astype(np.float64)) for i in range(T)])
    bw = np.stack([_softmax(layer_beta[i].astype(np.float64)) for i in range(T)])

    key = (round(L, 10), aw.tobytes(), bw.tobytes())
    if key not in _CACHE:
        _CACHE[key] = _build(L, aw, bw)
    nc = _CACHE[key]

    ident_mm = np.eye(128, dtype=mybir.dt.np(_store_dt(MM_DT)))
    xs = x[:, :, 0]                                        # [B, N]
    in_maps = [{
        "x": np.ascontiguousarray(xs[c * BS:(c + 1) * BS, :]),
        "w": np.ascontiguousarray(W),
        "ident_mm": ident_mm,
    } for c in range(NCORES)]

    res = run_bass_kernel_spmd(nc, in_maps, list(range(NCORES)),
                               trace=_want_trace)
    z = np.concatenate([res.results[c]["z_out"] for c in range(NCORES)], axis=1)
    out = np.ascontiguousarray(z.T)[:, :, None].astype(np.float32)
    if _want_trace:
        return out, res
    return out


if __name__ == "__main__":
    d = np.load('/tmp/inputs.npz')
    out = kernel(d['x'], d['frozen_weight'], d['alpha'], d['layer_beta'])
    ref = np.load('/tmp/ref_out.npy')
    rel = np.linalg.norm(out - ref) / np.linalg.norm(ref)
    print("rel err vs ref:", rel, "absmax:", np.abs(out - ref).max())


# revision 24
# speedup vs baseline: 1.4302x; 1.4302x over previous
"""Trainium2 Bass kernel for nn_DARTSModelLayers (FISTA-style unrolled model).

Math (per reference):
  W = frozen_weight[0]  [N=512, H=1024];  L = ||W||_2^2;  lam = 0.001/L
  10 iterations of:
    z_aux = z + (i/(i+3)) (z - z_prev)
    z_g   = z_aux - W^T(W z_aux - x)/L  =  (I - W^T W / L) z_aux + W^T x / L
    z_op  = sum_k softmax(alpha_i)_k * op_k(z_g)        (20 activations)
    z_prev = bw0 z + bw1 z_op ; z = z_op
  Re-expressed with host-folded scalars so each iteration is:
    tmp  = z_op_{i-2} * (coef_zold/coef_op) + z_op_{i-1}   (1 DVE pass)
    psum = M_noI @ tmp + I @ tmp                            (PE; M = I + M_noI)
    z_g  = psum * coef_op + c'                              (1 DVE pass)
    z_op = S_i(z_g)   via basis decomposition (ACT LUTs + DVE chains)

  S_i decomposition (weights w = softmax(alpha_i), see golden.py):
    basis: sigmoid(-x), tanh(x), erf(x/sqrt2), ln(sigmoid(-x)), exp(min(x,0)),
           sigmoid(-x)^2, |x|, clip(x,-1,1), clip(x/6+.5,0,1),
           1/(1+sm^2), 1/(1+|x|)
    z_op = x*V + sum_k cW_k * U_k, V = c_v0 + sum_k cV_k * T_k
    softshrink/hardshrink are approximated by identity (lam ~ 3.5e-4; max
    output error ~1e-4, validated against the jax reference in golden.py).

Sharding: batch B=4096 split over 8 cores (512 each); W/alpha/beta replicated.
M_noI = -W^T W/L and c' = W^T x/L are computed on-device; the host only
supplies the spectral norm L, softmax weights, and identity constants.
Output is produced in [H, B_shard] layout; the host transposes to [B, H, 1].
"""
import sys
import numpy as np

sys.path.insert(0, "/opt/trn_rl_repo")

import concourse.bass as bass  # noqa: E402
import concourse.bacc as bacc  # noqa: E402
import concourse.tile as tile  # noqa: E402
from concourse import mybir  # noqa: E402
from concourse.bass_utils import run_bass_kernel_spmd  # noqa: E402
from contextlib import ExitStack  # noqa: E402

F32 = mybir.dt.float32
F32R = mybir.dt.float32r
ACT = mybir.ActivationFunctionType
ALU = mybir.AluOpType

B, N, H, T = 4096, 512, 1024, 10
NCORES = 8
BS = B // NCORES          # 512 batch per core
NG = H // 128             # 8 h-tile groups
INV_SQRT2 = 0.7071067811865476
LAM_SELU = 1.0507009873554805
ALPHA_SELU = 1.6732632423543772

# ---- tuning switches ----
MM_DT = mybir.dt.float16      # per-iteration matmul dtype
SETUP_MM_DT = mybir.dt.float16  # setup (W^T W, W^T x) matmul dtype
CHAIN_DT = mybir.dt.float16   # basis/elementwise chain dtype
ACT_BATCH = 8                 # groups per ACT table-set batch
TRACE = False                 # set by test harness
DEBUG_DUMP = False            # add intermediate ExternalOutputs


def _store_dt(dt):
    # float32r tiles are real-typed: the BIR verifier requires matmul inputs
    # to be produced (rounded) as float32r, so no bitcasting.
    return dt


def _softmax(v):
    v = v - v.max()
    e = np.exp(v)
    return e / e.sum()


def _build(L, aw, bw, t_override=None):
    """Build the Bass program. aw [T,20], bw [T,2] host floats."""
    nc = bacc.Bacc("TRN2", target_bir_lowering=False, debug=False,
                   num_devices=NCORES)
    mm_st = _store_dt(MM_DT)
    su_st = _store_dt(SETUP_MM_DT)

    x_d = nc.dram_tensor("x", [BS, N], F32, kind="ExternalInput")
    w_d = nc.dram_tensor("w", [N, H], F32, kind="ExternalInput")
    im_d = nc.dram_tensor("ident_mm", [128, 128], mm_st, kind="ExternalInput")
    z_d = nc.dram_tensor("z_out", [H, BS], CHAIN_DT, kind="ExternalOutput")
    dbg = {}
    if DEBUG_DUMP:
        for nm, shp in (("c_sb", [128, NG * BS]), ("m_sb", [128, NG * H]),
                        ("z0", [128, NG * BS]), ("zg1", [128, NG * BS]),
                        ("xT", [128, 4 * BS])):
            dt = _store_dt(MM_DT) if nm == "m_sb" else F32
            dt = _store_dt(SETUP_MM_DT) if nm == "xT" else dt
            dbg[nm] = nc.dram_tensor("dbg_" + nm, shp, dt,
                                     kind="ExternalOutput")

    invL = 1.0 / L

    with tile.TileContext(nc) as tc, ExitStack() as ctx:
        ctx.enter_context(nc.allow_low_precision(
            reason="fp16 basis chain; error validated against jax reference"))
        state = ctx.enter_context(tc.tile_pool(name="state", bufs=1))
        # 8 persistent psum tiles (one bank each) -- no pool cycling, so no
        # SP release waits land on matmuls (walrus: max 1 wait per Matmult)
        psfix = ctx.enter_context(tc.tile_pool(name="psfix", bufs=1,
                                               space="PSUM"))
        ps_fix = [psfix.tile([128, BS], F32, name=f"psf{g}") for g in range(NG)]
        zA = state.tile([128, NG * BS], CHAIN_DT, name="zA")
        zB = state.tile([128, NG * BS], CHAIN_DT, name="zB")
        zg = state.tile([128, NG * BS], F32, name="zg")
        c_sb = state.tile([128, NG * BS], CHAIN_DT, name="c_sb")
        m_sb = state.tile([128, NG * H], mm_st, name="m_sb")
        ident_mm = state.tile([128, 128], mm_st, name="ident_mm")
        nc.sync.dma_start(ident_mm[:], im_d[:, :])

        # ---------------- setup: M_noI and c' ----------------
        with tc.tile_pool(name="setup", bufs=1) as sp:
            w_sb = sp.tile([128, 4 * H], F32, name="w_sb")
            w_rhs = sp.tile([128, 4 * H], su_st, name="w_rhs")
            w_lhs = sp.tile([128, 4 * H], su_st, name="w_lhs")
            x_sb = sp.tile([128, 4 * N], F32, name="x_sb")
            x16 = sp.tile([128, 4 * N], F32, name="x16")
            xT_sb = sp.tile([128, 4 * BS], su_st, name="xT_sb")
            xT2 = sp.tile([128, 4 * BS], su_st, name="xT2")
            # single DMA per tensor so downstream consumers carry one wait
            nc.sync.dma_start(w_sb[:].rearrange("p (j h) -> p j h", j=4),
                              w_d[:, :].rearrange("(j p) h -> p j h", p=128))
            nc.sync.dma_start(x_sb[:].rearrange("p (j n) -> p j n", j=4),
                              x_d[:, :].rearrange("(j p) n -> p j n", p=128))
            # rhs for A-matmul: -W/L (DVE); lhs copy of W (ACT)
            nc.vector.tensor_scalar(w_rhs[:], w_sb[:], -invL, None, ALU.mult)
            nc.scalar.copy(w_lhs[:], w_sb[:])
            # x/L in f32, PE-transpose 128x128 blocks to [n, b] layout, cast
            # to fp16 on evacuation. Both transpose inputs come from ACT so
            # each transpose carries a single (ACT) wait.
            nc.scalar.activation(x16[:], x_sb[:], ACT.Copy, scale=invL)
            identf_a = sp.tile([128, 128], F32, name="identf_a")
            nc.scalar.copy(identf_a[:], ident_mm[:])
            for bj in range(4):
                for nk in range(4):
                    pst = ps_fix[bj]
                    nc.tensor.transpose(
                        pst[:, 0:128],
                        x16[:, bj * N + nk * 128: bj * N + nk * 128 + 128],
                        identf_a[:])
                    nc.vector.tensor_scalar(
                        xT_sb[:, nk * BS + bj * 128: nk * BS + bj * 128 + 128],
                        pst[:, 0:128], 1.0, None, ALU.mult)
            # funnel the 16 transpose evacs through one ACT producer so the
            # cc-matmuls carry a single (ACT) wait
            nc.scalar.copy(xT2[:], xT_sb[:])
            if DEBUG_DUMP:
                nc.sync.dma_start(dbg["xT"][:, :], xT2[:])

            # M_noI = -W^T W / L   -> m_sb[h1-part(g), h2-free]
            for g in range(NG):
                for half in range(2):
                    ps = ps_fix[g]
                    for j in range(4):
                        nc.tensor.matmul(
                            ps[:, half * 0: 512],
                            w_lhs[:, j * H + g * 128: j * H + g * 128 + 128],
                            w_rhs[:, j * H + half * 512: j * H + half * 512 + 512],
                            start=(j == 0), stop=(j == 3))
                    nc.scalar.copy(
                        m_sb[:, g * H + half * 512: g * H + half * 512 + 512],
                        ps[:, 0:512])

            # c' = W^T x / L  -> c_sb[h-part(g), b-free]
            for g in range(NG):
                ps = ps_fix[g]
                for nk in range(4):
                    nc.tensor.matmul(
                        ps[:],
                        w_lhs[:, nk * H + g * 128: nk * H + g * 128 + 128],
                        xT2[:, nk * BS:(nk + 1) * BS],
                        start=(nk == 0), stop=(nk == 3))
                # evacuate via DVE so iteration-1 matmuls' WAR on this bank
                # merges with their DVE wait on tmp
                nc.vector.tensor_scalar(c_sb[:, g * BS:(g + 1) * BS], ps[:],
                                        1.0, None, ALU.mult)

            nc.vector.memset(zB[:], 0.0)

        # ---------------- iterations ----------------
        actp = ctx.enter_context(tc.tile_pool(name="actb", bufs=1))
        dvep = ctx.enter_context(tc.tile_pool(name="dveb", bufs=1))
        tmp = state.tile([128, NG * BS], mm_st, name="tmp")
        xh_t = (state.tile([128, NG * BS], CHAIN_DT, name="xh_t")
                if CHAIN_DT != F32 else None)

        z_im1, z_im2 = None, zB   # z_op_{i-1}, z_op_{i-2}
        T_eff = T if t_override is None else t_override
        for i in range(T_eff):
            w = aw[i]
            c_r = w[1] + 0.99 * w[10] + w[4] + w[9] + LAM_SELU * w[8]
            wE = w[4] + w[9] + LAM_SELU * ALPHA_SELU * w[8]
            K = w[16] - wE
            c_v0 = (w[2] + w[11] + w[12] + 0.5 * w[3] + w[18]
                    + 0.01 * w[10] + 0.5 * c_r + w[0] + w[5])
            cV = {"e2": 0.5 * w[3], "sm": -w[18], "th": w[19], "hm": w[7]}
            cW = {"A": 0.5 * c_r, "t": w[15] - w[12], "sm": -w[16],
                  "ln": w[11] - w[14], "E": wE, "c1": w[6], "hm": w[17],
                  "ss": w[13]}

            if i == 0:
                x_src = c_sb
            else:
                mom = i / (i + 3.0)
                bwp = bw[i - 1]
                coef_op = 1.0 + mom * (1.0 - bwp[1])
                coef_zold = -mom * bwp[0]
                nc.vector.scalar_tensor_tensor(
                    tmp[:], z_im2[:], coef_zold / coef_op, z_im1[:],
                    ALU.mult, ALU.add)
                for g in range(NG):
                    ps = ps_fix[g]
                    for j in range(NG):
                        nc.tensor.matmul(
                            ps[:],
                            m_sb[:, j * H + g * 128: j * H + g * 128 + 128],
                            tmp[:, j * BS:(j + 1) * BS],
                            start=(j == 0), stop=False)
                    nc.tensor.matmul(ps[:], ident_mm[:],
                                     tmp[:, g * BS:(g + 1) * BS],
                                     start=False, stop=True)
                    nc.vector.scalar_tensor_tensor(
                        zg[:, g * BS:(g + 1) * BS], ps[:], coef_op,
                        c_sb[:, g * BS:(g + 1) * BS], ALU.mult, ALU.add)
                x_src = zg

            z_out_t = zA if i % 2 == 0 else zB

            # fp16 copy of x for the DVE-side chain (ACT does the cast)
            xh = xh_t
            FS = NG * BS   # full free size

            def full(t):
                return t[:, 0:FS]

            # ---- ACT basis, batched by table set ----
            # B1: sigmoid_and_others (sigmoid, erf, abs, copy)
            nc.scalar.copy(full(xh), full(x_src))
            sm = actp.tile([128, FS], CHAIN_DT, tag="sm", name="sm")
            nc.scalar.activation(full(sm), full(x_src), ACT.Sigmoid, scale=-1.0)
            e2 = actp.tile([128, FS], CHAIN_DT, tag="e2", name="e2")
            nc.scalar.activation(full(e2), full(x_src), ACT.Erf, scale=INV_SQRT2)
            Aq = actp.tile([128, FS], CHAIN_DT, tag="Aq", name="Aq")
            nc.scalar.activation(full(Aq), full(x_src), ACT.Abs)
            # DVE helper needed before Exp
            m0 = dvep.tile([128, FS], CHAIN_DT, tag="h1", name="m0")
            nc.vector.tensor_scalar(full(m0), full(xh), 0.0, None, ALU.min)
            # B2: natural_log_exp_and_others (ln, exp)
            lnsm = actp.tile([128, FS], CHAIN_DT, tag="lnsm", name="lnsm")
            nc.scalar.activation(full(lnsm), full(sm), ACT.Ln)
            Et = actp.tile([128, FS], CHAIN_DT, tag="Et", name="Et")
            nc.scalar.activation(full(Et), full(m0), ACT.Exp)
            # B3: exp_and_others (tanh) -- tanh(x) and tanh(softplus)=tanh(-lnsm)
            tt = actp.tile([128, FS], CHAIN_DT, tag="tt", name="tt")
            nc.scalar.activation(full(tt), full(x_src), ACT.Tanh)
            th = actp.tile([128, FS], CHAIN_DT, tag="th", name="th")
            nc.scalar.activation(full(th), full(lnsm), ACT.Tanh, scale=-1.0)

            # ---- DVE helpers (fp16, 4x TS) ----
            def dv(tag, nm):
                return dvep.tile([128, FS], CHAIN_DT, tag=tag, name=nm)

            c1 = dv("h2", "c1")
            nc.vector.tensor_scalar(full(c1), full(xh), 1.0, -1.0,
                                    ALU.min, ALU.max)
            c1k = dv("c1k", "c1k")
            nc.vector.tensor_scalar(full(c1k), full(c1), K / cW["c1"], None,
                                    ALU.add)
            hm1 = dv("h3", "hm1")
            nc.vector.tensor_scalar(full(hm1), full(xh), 1.0 / 6.0, 0.5,
                                    ALU.mult, ALU.add)
            hm = dv("hm", "hm")
            nc.vector.tensor_scalar(full(hm), full(hm1), 1.0, 0.0,
                                    ALU.min, ALU.max)
            d2 = dv("h1", "d2")
            nc.vector.tensor_scalar(full(d2), full(Aq), 1.0, None, ALU.add)
            # softsign = x / (1+|x|):  r2 = (1/sqrt(d2))^2 on ACT, then x*r2
            rq = actp.tile([128, FS], CHAIN_DT, tag="rq", name="rq")
            nc.scalar.activation(full(rq), full(d2), ACT.Abs_reciprocal_sqrt)
            r2 = actp.tile([128, FS], CHAIN_DT, tag="rq2", name="rq2")
            nc.scalar.activation(full(r2), full(rq), ACT.Square)
            ss = dv("h2", "ss")
            nc.vector.tensor_mul(full(ss), full(xh), full(r2))

            # ---- V chain: V = c_e2*e2 + c_v0 + c_sm*sm + c_th*th + c_hm*hm
            V = dv("V0", "V")
            nc.vector.tensor_scalar(full(V), full(e2), cV["e2"], c_v0,
                                    ALU.mult, ALU.add)
            for tag, (tens, cc_) in zip(
                    ("V1", "V0", "V1"),
                    ((sm, cV["sm"]), (th, cV["th"]), (hm, cV["hm"]))):
                Vn = dv(tag, "Vn")
                nc.vector.scalar_tensor_tensor(full(Vn), full(tens), cc_,
                                               full(V), ALU.mult, ALU.add)
                V = Vn
            # acc alternates between the two V tiles (V is dead after x*V)
            acc = dv("V0" if V.tensor.name.startswith("Vn") else "V1", "acc")
            nc.vector.tensor_mul(full(acc), full(xh), full(V))
            chain = [(Aq, cW["A"]), (tt, cW["t"]), (sm, cW["sm"]),
                     (lnsm, cW["ln"]), (Et, cW["E"]), (c1k, cW["c1"]),
                     (ss, cW["ss"]), (hm, cW["hm"])]
            for idx, (tens, cc_) in enumerate(chain):
                if idx == len(chain) - 1:
                    nc.vector.scalar_tensor_tensor(
                        full(z_out_t), full(tens), cc_, full(acc),
                        ALU.mult, ALU.add)
                else:
                    nacc = dv("V1" if idx % 2 == 0 else "V0", "acc")
                    nc.vector.scalar_tensor_tensor(
                        full(nacc), full(tens), cc_, full(acc),
                        ALU.mult, ALU.add)
                    acc = nacc

            z_im2 = z_im1 if z_im1 is not None else zB
            z_im1 = z_out_t

        if DEBUG_DUMP:
            nc.sync.dma_start(dbg["c_sb"][:, :], c_sb[:])
            nc.sync.dma_start(dbg["m_sb"][:, :], m_sb[:])
            nc.sync.dma_start(dbg["z0"][:, :], zA[:])
            nc.sync.dma_start(dbg["zg1"][:, :], zg[:])
        # output
        for g in range(NG):
            nc.sync.dma_start(z_d[g * 128:(g + 1) * 128, :],
                              z_im1[:, g * BS:(g + 1) * BS])

    nc.finalize()
    return nc


_CACHE = {}


def kernel(x, frozen_weight, alpha, layer_beta, _want_trace=False):
    x = np.asarray(x, np.float32)
    frozen_weight = np.asarray(frozen_weight, np.float32)
    alpha = np.asarray(alpha, np.float32)
    layer_beta = np.asarray(layer_beta, np.float32)

    W = frozen_weight[0]                                   # [N, H]
    L = float(np.linalg.norm(W.astype(np.float64), 2) ** 2)
    aw = np.stack([_softmax(alpha[i].astype(np.float64)) for i in range(T)])
    bw = np.stack([_softmax(layer_beta[i].astype(np.float64)) for i in range(T)])

    key = (round(L, 10), aw.tobytes(), bw.tobytes())
    if key not in _CACHE:
        _CACHE[key] = _build(L, aw, bw)
    nc = _CACHE[key]

    ident_mm = np.eye(128, dtype=mybir.dt.np(_store_dt(MM_DT)))
    xs = x[:, :, 0]                                        # [B, N]
    in_maps = [{
        "x": np.ascontiguousarray(xs[c * BS:(c + 1) * BS, :]),
        "w": np.ascontiguousarray(W),
        "ident_mm": ident_mm,
    } for c in range(NCORES)]

    res = run_bass_kernel_spmd(nc, in_maps, list(range(NCORES)),
                               trace=_want_trace)
    z = np.concatenate([np.asarray(res.results[c]["z_out"], np.float32)
                        for c in range(NCORES)], axis=1)
    out = np.ascontiguousarray(z.T)[:, :, None].astype(np.float32)
    if _want_trace:
        return out, res
    return out


if __name__ == "__main__":
    d = np.load('/tmp/inputs.npz')
    out = kernel(d['x'], d['frozen_weight'], d['alpha'], d['layer_beta'])
    ref = np.load('/tmp/ref_out.npy')
    rel = np.linalg.norm(out - ref) / np.linalg.norm(ref)
    print("rel err vs ref:", rel, "absmax:", np.abs(out - ref).max())
